# revision 9
# baseline (speedup 1.0000x reference)
"""Bass/Tile Trainium2 kernel for the CAFBlock fusion (nn_CAFBlock).

Strategy: shard the audio channel dim C_a=128 across 8 NeuronCores (16
channels per core).  BatchNorm2d statistics are per-channel -> fully local.
The tiny video branch (gLN over all channels) is computed redundantly on
every core from a replicated copy of v1, so there are no collectives.

Per-core SBUF layout for the big tensors: partition p = b*64 + k where k is
the video-frame index (t = k*8 + r), free dim = (c_local, r, f).  With this
layout the nearest-interpolated v_attn/v_key factors are constant along the
free dim, so each fused output tile needs only per-partition [128,1] scalar
operands:
    out[ns,c] = a1*(Av*attn) + (Bv*attn) + v_key * relu(a1*Ag + Bg)
computed as one ACT/DVE affine op + one scalar_tensor_tensor op.
"""

import numpy as np

import concourse.bass as bass
import concourse.bacc as bacc
import concourse.tile as tile
import concourse.mybir as mybir
from concourse.bass_utils import run_bass_kernel_spmd

F32 = mybir.dt.float32
AF = mybir.ActivationFunctionType
OP = mybir.AluOpType
AX = mybir.AxisListType
MS = bass.MemorySpace

# problem dims (hardcoded per the harness contract)
B, NS, CA, H, T, FQ, TV = 2, 2, 128, 4, 512, 128, 64
NCORE = 8
CL = CA // NCORE            # 16 local channels per core
N = B * NS                  # 4 (b*ns video samples)
RP = T // TV                # 8 (nearest-interp repeat factor)
BN_EPS, GLN_EPS = 1e-5, 1e-8
NBN = float(B * T * FQ)     # 131072 elements per BN channel
NKEY = float(CA * TV)       # 8192 elements per gLN(key) sample
NATT = float(CA * H * TV)   # 32768 elements per gLN(attn) sample
CF = RP * FQ                # 1024 free elements per channel tile
AFREE = CL * CF             # 16384 free elements of resident a1 shard
OFREE = CL * NS * CF        # 32768 free elements of output


def _build():
    """Builds the SPMD Bass program (same program on all 8 cores)."""
    nc = bacc.Bacc("TRN2", target_bir_lowering=False, debug=False)

    d_a1 = nc.dram_tensor("a1s", [128, AFREE], F32, kind="ExternalInput")
    d_v1f = nc.dram_tensor("v1f", [128, N * TV], F32, kind="ExternalInput")
    d_v1l = nc.dram_tensor("v1l", [CL, N * TV], F32, kind="ExternalInput")
    d_pcol = nc.dram_tensor("pcol", [128, 10], F32, kind="ExternalInput")
    d_ploc = nc.dram_tensor("ploc", [CL, 18], F32, kind="ExternalInput")
    d_prow = nc.dram_tensor("prow", [1, 96], F32, kind="ExternalInput")
    d_onec = nc.dram_tensor("onec", [128, 1], F32, kind="ExternalInput")
    d_oner = nc.dram_tensor("oner", [1, 128], F32, kind="ExternalInput")
    d_id16 = nc.dram_tensor("id16", [CL, CL], F32, kind="ExternalInput")
    d_out = nc.dram_tensor("out", [128, OFREE], F32, kind="ExternalOutput")

    with tile.TileContext(nc) as tc:
        with (
            tc.tile_pool(name="pres", bufs=8) as pres,
            tc.tile_pool(name="pconst", bufs=1) as pc,
            tc.tile_pool(name="pgate", bufs=2) as pgate,
            tc.tile_pool(name="ps0", bufs=2) as ps0,
            tc.tile_pool(name="ps1", bufs=2) as ps1,
            tc.tile_pool(name="pout", bufs=3) as pout,
            tc.tile_pool(name="pps", bufs=1, space=MS.PSUM) as pps,
            tc.tile_pool(name="ppt", bufs=2, space=MS.PSUM) as ppt,
            tc.tile_pool(name="ppb", bufs=1, space=MS.PSUM) as ppb,
        ):
            # ---------------- input DMAs (HWDGE, 1MB each) ----------------
            res = []
            for g in range(8):
                t = pres.tile([128, 2048], F32, tag="res")
                nc.sync.dma_start(t[:], d_a1.ap()[:, g * 2048:(g + 1) * 2048])
                res.append(t)

            def a1c(c):
                return res[c // 2][:, (c % 2) * CF:(c % 2) * CF + CF]

            # ---------------- constants (SWDGE ring) ----------------------
            v1f = pc.tile([128, N * TV], F32, tag="v1f")
            v1l = pc.tile([CL, N * TV], F32, tag="v1l")
            pcol = pc.tile([128, 10], F32, tag="pcol")
            ploc = pc.tile([CL, 18], F32, tag="ploc")
            prow = pc.tile([1, 96], F32, tag="prow")
            onec = pc.tile([128, 1], F32, tag="onec")
            oner = pc.tile([1, 128], F32, tag="oner")
            id16 = pc.tile([CL, CL], F32, tag="id16")
            for tl, dr in ((v1f, d_v1f), (v1l, d_v1l), (pcol, d_pcol),
                           (ploc, d_ploc), (prow, d_prow), (onec, d_onec),
                           (oner, d_oner), (id16, d_id16)):
                nc.gpsimd.dma_start(tl[:], dr.ap()[:])

            # ---------------- BN stats (pipelined with input DMA) ---------
            # bn_stats per channel chunk: [128, 2, 512] -> [128, 2, 6]
            stats = pc.tile([128, CL * 12], F32, tag="stats")
            for c in range(CL):
                for k in range(2):
                    nc.vector.bn_stats(
                        stats[:, (c * 2 + k) * 6:(c * 2 + k + 1) * 6],
                        a1c(c)[:, k * 512:(k + 1) * 512],
                    )
            # mean^2 (needed before cross-partition sum)
            msq = pc.tile([128, CL * 4], F32, tag="msq")
            sv = stats[:].rearrange("p (c k h s) -> p c k h s",
                                    c=CL, k=2, h=2, s=3)[:, :, :, :, 1]
            mv = msq[:].rearrange("p (c k h) -> p c k h", c=CL, k=2, h=2)
            nc.vector.tensor_tensor(mv, sv, sv, OP.mult)

            # ---------------- v-branch full-channel stats -----------------
            # key: vk = v1*wk + bk ; per-sample sums over (c, tv)
            vkf = pc.tile([128, N * TV], F32, tag="vkf")
            nc.vector.tensor_scalar(vkf[:], v1f[:], pcol[:, 0:1], pcol[:, 1:2],
                                    OP.mult, OP.add)
            ks = pc.tile([128, 8], F32, tag="ks")
            nc.vector.tensor_reduce(
                ks[:, 0:4], vkf[:].rearrange("p (n t) -> p n t", n=N, t=TV),
                axis=AX.X, op=OP.add)
            scrk = pc.tile([128, N * TV], F32, tag="scrk")
            nc.vector.tensor_tensor(scrk[:], vkf[:], vkf[:], OP.mult)
            nc.vector.tensor_reduce(
                ks[:, 4:8], scrk[:].rearrange("p (n t) -> p n t", n=N, t=TV),
                axis=AX.X, op=OP.add)

            # attn: va[h] = v1*wa_h + ba_h ; per-sample sums over (c, h, tv)
            va = pc.tile([128, H * N * TV], F32, tag="va")
            for h in range(H):
                nc.vector.tensor_scalar(
                    va[:, h * N * TV:(h + 1) * N * TV], v1f[:],
                    pcol[:, 2 + h:3 + h], pcol[:, 6 + h:7 + h], OP.mult, OP.add)
            asum = pc.tile([128, 8], F32, tag="asum")
            nc.vector.tensor_reduce(
                asum[:, 0:4],
                va[:].rearrange("p (h n t) -> p n h t", h=H, n=N, t=TV),
                axis=AX.XY, op=OP.add)
            scra = pc.tile([128, H * N * TV], F32, tag="scra")
            nc.vector.tensor_tensor(scra[:], va[:], va[:], OP.mult)
            nc.vector.tensor_reduce(
                asum[:, 4:8],
                scra[:].rearrange("p (h n t) -> p n h t", h=H, n=N, t=TV),
                axis=AX.XY, op=OP.add)

            # ---------------- cross-partition reduction (PE) --------------
            pp_st = pps.tile([1, CL * 12], F32, tag="ppst")
            pp_mq = pps.tile([1, CL * 4], F32, tag="ppmq")
            pp_ks = pps.tile([1, 8], F32, tag="ppks")
            pp_as = pps.tile([1, 8], F32, tag="ppas")
            nc.tensor.matmul(pp_st[:], onec[:], stats[:], start=True, stop=True)
            nc.tensor.matmul(pp_mq[:], onec[:], msq[:], start=True, stop=True)
            nc.tensor.matmul(pp_ks[:], onec[:], ks[:], start=True, stop=True)
            nc.tensor.matmul(pp_as[:], onec[:], asum[:], start=True, stop=True)
            st = pc.tile([1, CL * 12], F32, tag="st")
            mq = pc.tile([1, CL * 4], F32, tag="mq")
            kr = pc.tile([1, 8], F32, tag="kr")
            ar = pc.tile([1, 8], F32, tag="ar")
            nc.vector.tensor_copy(st[:], pp_st[:])
            nc.vector.tensor_copy(mq[:], pp_mq[:])
            nc.vector.tensor_copy(kr[:], pp_ks[:])
            nc.vector.tensor_copy(ar[:], pp_as[:])

            # ---------------- finalize rows (partition 0, all tiny) -------
            # rw layout: 0:16 msum | 16:32 cvsum | 32:48 mqsum | 48:64 mean
            # 64:80 ex2 | 80:96 u1 | 96:112 u2 | 112:128 mts | 128:144 var
            rw = pc.tile([1, 144], F32, tag="rw")
            stv = st[:].rearrange("p (c k h s) -> p c k h s",
                                  c=CL, k=2, h=2, s=3)
            nc.vector.tensor_reduce(rw[:, 0:16], stv[:, :, :, :, 1],
                                    axis=AX.XY, op=OP.add)
            nc.vector.tensor_reduce(rw[:, 16:32], stv[:, :, :, :, 2],
                                    axis=AX.XY, op=OP.add)
            nc.vector.tensor_reduce(
                rw[:, 32:48], mq[:].rearrange("p (c g) -> p c g", c=CL, g=4),
                axis=AX.X, op=OP.add)
            # mean = msum/512 ; ex2 = cvsum/131072 + mqsum/512 ; var = ex2-m^2
            nc.vector.tensor_scalar_mul(rw[:, 48:64], rw[:, 0:16], 1.0 / 512.0)
            nc.vector.tensor_scalar_mul(rw[:, 80:96], rw[:, 16:32], 1.0 / NBN)
            nc.vector.tensor_scalar_mul(rw[:, 96:112], rw[:, 32:48], 1.0 / 512.0)
            nc.vector.tensor_tensor(rw[:, 64:80], rw[:, 80:96], rw[:, 96:112],
                                    OP.add)
            nc.vector.tensor_tensor(rw[:, 112:128], rw[:, 48:64], rw[:, 48:64],
                                    OP.mult)
            nc.vector.tensor_tensor(rw[:, 128:144], rw[:, 64:80],
                                    rw[:, 112:128], OP.subtract)

            # v rows: kv layout 0:4 kmean | 4:8 kex2 | 8:12 kvar | 12:16 km^2
            #         16:20 amean | 20:24 aex2 | 24:28 avar | 28:32 am^2
            kv = pc.tile([1, 32], F32, tag="kv")
            nc.vector.tensor_scalar_mul(kv[:, 0:4], kr[:, 0:4], 1.0 / NKEY)
            nc.vector.tensor_scalar_mul(kv[:, 4:8], kr[:, 4:8], 1.0 / NKEY)
            nc.vector.tensor_tensor(kv[:, 12:16], kv[:, 0:4], kv[:, 0:4],
                                    OP.mult)
            nc.vector.tensor_tensor(kv[:, 8:12], kv[:, 4:8], kv[:, 12:16],
                                    OP.subtract)
            nc.vector.tensor_scalar_mul(kv[:, 16:20], ar[:, 0:4], 1.0 / NATT)
            nc.vector.tensor_scalar_mul(kv[:, 20:24], ar[:, 4:8], 1.0 / NATT)
            nc.vector.tensor_tensor(kv[:, 28:32], kv[:, 16:20], kv[:, 16:20],
                                    OP.mult)
            nc.vector.tensor_tensor(kv[:, 24:28], kv[:, 20:24], kv[:, 28:32],
                                    OP.subtract)

            # q row [1,40]: var_val*wv^2+eps | var_gate*wg^2+eps | qkey | qattn
            # prow layout: wv 0:16 | gv 16:32 | bev 32:48 | wg 48:64
            #              gg 64:80 | beg 80:96
            q = pc.tile([1, 40], F32, tag="q")
            w2 = pc.tile([1, 32], F32, tag="w2")
            nc.vector.tensor_tensor(w2[:, 0:16], prow[:, 0:16], prow[:, 0:16],
                                    OP.mult)
            nc.vector.tensor_tensor(w2[:, 16:32], prow[:, 48:64],
                                    prow[:, 48:64], OP.mult)
            nc.vector.tensor_tensor(q[:, 0:16], rw[:, 128:144], w2[:, 0:16],
                                    OP.mult)
            nc.vector.tensor_tensor(q[:, 16:32], rw[:, 128:144], w2[:, 16:32],
                                    OP.mult)
            nc.vector.tensor_scalar_add(q[:, 0:32], q[:, 0:32], BN_EPS)
            nc.vector.tensor_scalar_add(q[:, 32:36], kv[:, 8:12], GLN_EPS)
            nc.vector.tensor_scalar_add(q[:, 36:40], kv[:, 24:28], GLN_EPS)

            # rs = 1/sqrt(q) via exp(-0.5*ln(q)) + one Newton polish
            lnq = pc.tile([1, 40], F32, tag="lnq")
            rs0 = pc.tile([1, 40], F32, tag="rs0")
            rs = pc.tile([1, 40], F32, tag="rs")
            nt = pc.tile([1, 40], F32, tag="nt")
            nc.scalar.activation(lnq[:], q[:], AF.Ln)
            nc.scalar.activation(rs0[:], lnq[:], AF.Exp, scale=-0.5)
            nc.vector.tensor_tensor(nt[:], rs0[:], rs0[:], OP.mult)
            nc.vector.tensor_tensor(nt[:], q[:], nt[:], OP.mult)
            nc.vector.tensor_scalar(nt[:], nt[:], -1.0, 3.0, OP.mult, OP.add)
            nc.vector.tensor_scalar_mul(rs[:], rs0[:], 0.5)
            nc.vector.tensor_tensor(rs[:], rs[:], nt[:], OP.mult)

            # Av/Bv/Ag/Bg row [1,64]
            ab = pc.tile([1, 64], F32, tag="ab")
            nc.vector.tensor_tensor(ab[:, 0:16], rs[:, 0:16], prow[:, 16:32],
                                    OP.mult)
            nc.vector.tensor_tensor(ab[:, 0:16], ab[:, 0:16], prow[:, 0:16],
                                    OP.mult)
            nc.vector.tensor_tensor(ab[:, 16:32], rw[:, 48:64], ab[:, 0:16],
                                    OP.mult)
            nc.vector.tensor_tensor(ab[:, 16:32], prow[:, 32:48], ab[:, 16:32],
                                    OP.subtract)
            nc.vector.tensor_tensor(ab[:, 32:48], rs[:, 16:32], prow[:, 64:80],
                                    OP.mult)
            nc.vector.tensor_tensor(ab[:, 32:48], ab[:, 32:48], prow[:, 48:64],
                                    OP.mult)
            nc.vector.tensor_tensor(ab[:, 48:64], rw[:, 48:64], ab[:, 32:48],
                                    OP.mult)
            nc.vector.tensor_tensor(ab[:, 48:64], prow[:, 80:96], ab[:, 48:64],
                                    OP.subtract)

            # bc1 row [1,28]: kmean(4) | rs_key(4) | amean*rs_attn(4) |
            #                 rs_attn repeated n-major h-minor (16)
            b1 = pc.tile([1, 28], F32, tag="b1")
            nc.vector.tensor_copy(b1[:, 0:4], kv[:, 0:4])
            nc.vector.tensor_copy(b1[:, 4:8], rs[:, 32:36])
            nc.vector.tensor_tensor(b1[:, 8:12], kv[:, 16:20], rs[:, 36:40],
                                    OP.mult)
            b1rep = b1[:, 12:28].rearrange("p (n x) -> p n x", n=N, x=H)
            rsat = rs[:, 36:40].rearrange("p (n x) -> p n x", n=N, x=1)
            for h in range(H):
                nc.vector.tensor_copy(b1rep[:, :, h:h + 1], rsat[:])

            # broadcast rows across partitions (PE, K=1)
            pp_b1 = ppb.tile([128, 28], F32, tag="ppb1")
            pp_ab = ppb.tile([128, 64], F32, tag="ppab")
            nc.tensor.matmul(pp_b1[:], oner[:], b1[:], start=True, stop=True)
            nc.tensor.matmul(pp_ab[:], oner[:], ab[:], start=True, stop=True)
            bc1 = pc.tile([128, 28], F32, tag="bc1")
            bcab = pc.tile([128, 64], F32, tag="bcab")
            nc.vector.tensor_copy(bc1[:], pp_b1[:])
            nc.vector.tensor_copy(bcab[:], pp_ab[:])

            # ---------------- local v-branch ------------------------------
            # ploc: wk 0 | bk 1 | gk 2 | bek 3 | wa 4:8 | ba 8:12 |
            #       ga/4 12:16 | sum(ga)/4 16 | sum(bea)/4 17
            vkl = pc.tile([CL, N * TV], F32, tag="vkl")
            nc.vector.tensor_scalar(vkl[:], v1l[:], ploc[:, 0:1], ploc[:, 1:2],
                                    OP.mult, OP.add)
            kscol = pc.tile([CL, N], F32, tag="kscol")
            kbcol = pc.tile([CL, N], F32, tag="kbcol")
            for n in range(N):
                nc.vector.tensor_tensor(kscol[:, n:n + 1], ploc[:, 2:3],
                                        bc1[0:CL, 4 + n:5 + n], OP.mult)
                nc.vector.tensor_tensor(kbcol[:, n:n + 1], kscol[:, n:n + 1],
                                        bc1[0:CL, n:n + 1], OP.mult)
                nc.vector.tensor_tensor(kbcol[:, n:n + 1], ploc[:, 3:4],
                                        kbcol[:, n:n + 1], OP.subtract)
            # vkln/soft stored in (ns, b, tv) column order so the transpose
            # lhsT slice [16, 128] is contiguous: perm(n) = (n%2)*2 + n//2
            perm = [(n % 2) * 2 + n // 2 for n in range(N)]
            vkln = pc.tile([CL, N * TV], F32, tag="vkln")
            for n in range(N):
                nc.vector.tensor_scalar(
                    vkln[:, perm[n] * TV:(perm[n] + 1) * TV],
                    vkl[:, n * TV:(n + 1) * TV],
                    kscol[:, n:n + 1], kbcol[:, n:n + 1], OP.mult, OP.add)

            val = pc.tile([CL, H * N * TV], F32, tag="val")
            for h in range(H):
                nc.vector.tensor_scalar(
                    val[:, h * N * TV:(h + 1) * N * TV], v1l[:],
                    ploc[:, 4 + h:5 + h], ploc[:, 8 + h:9 + h], OP.mult, OP.add)
            ga16 = pc.tile([CL, N * H], F32, tag="ga16")
            for n in range(N):
                nc.vector.tensor_copy(ga16[:, n * H:(n + 1) * H],
                                      ploc[:, 12:16])
            sc16 = pc.tile([CL, N * H], F32, tag="sc16")
            nc.vector.tensor_tensor(sc16[:], ga16[:], bc1[0:CL, 12:28],
                                    OP.mult)
            bicol = pc.tile([CL, N], F32, tag="bicol")
            for n in range(N):
                nc.vector.tensor_tensor(bicol[:, n:n + 1], ploc[:, 16:17],
                                        bc1[0:CL, 8 + n:9 + n], OP.mult)
                nc.vector.tensor_tensor(bicol[:, n:n + 1], ploc[:, 17:18],
                                        bicol[:, n:n + 1], OP.subtract)
            # vm[n] = sum_h val[h,n]*sc16[n,h] + bicol[n]  (ga,bea host-/4)
            vm = pc.tile([CL, N * TV], F32, tag="vm")
            for n in range(N):
                dst = vm[:, n * TV:(n + 1) * TV]
                nc.vector.tensor_scalar(
                    dst, val[:, n * TV:n * TV + TV],
                    sc16[:, n * H:n * H + 1], bicol[:, n:n + 1],
                    OP.mult, OP.add)
                for h in range(1, H):
                    nc.vector.scalar_tensor_tensor(
                        dst, val[:, h * N * TV + n * TV:h * N * TV + n * TV + TV],
                        sc16[:, n * H + h:n * H + h + 1], dst, OP.mult, OP.add)
            # softmax over tv per (c, n)
            mx = pc.tile([CL, N], F32, tag="mx")
            nc.vector.tensor_reduce(
                mx[:], vm[:].rearrange("p (n t) -> p n t", n=N, t=TV),
                axis=AX.X, op=OP.max)
            nmx = pc.tile([CL, N], F32, tag="nmx")
            nc.vector.tensor_scalar_mul(nmx[:], mx[:], -1.0)
            ex = pc.tile([CL, N * TV], F32, tag="ex")
            ssum = pc.tile([CL, N], F32, tag="ssum")
            for n in range(N):
                nc.scalar.activation(
                    ex[:, n * TV:(n + 1) * TV], vm[:, n * TV:(n + 1) * TV],
                    AF.Exp, bias=nmx[:, n:n + 1],
                    accum_out=ssum[:, n:n + 1])
            rcp = pc.tile([CL, N], F32, tag="rcp")
            nc.vector.reciprocal(rcp[:], ssum[:])
            soft = pc.tile([CL, N * TV], F32, tag="soft")
            for n in range(N):
                nc.vector.tensor_scalar_mul(
                    soft[:, perm[n] * TV:(perm[n] + 1) * TV],
                    ex[:, n * TV:(n + 1) * TV], rcp[:, n:n + 1])

            # ---------------- transpose to (b,k) x (ns,c) -----------------
            # out[(b,tv), c] = src[c, (2b+ns)*TV + tv] via lhsT^T @ I16 with a
            # strided lhsT view gathering both b halves (M=128, K=16).
            tkey = pc.tile([128, NS * CL], F32, tag="tkey")
            tatt = pc.tile([128, NS * CL], F32, tag="tatt")
            for (src, dst) in ((vkln, tkey), (soft, tatt)):
                for ns in range(NS):
                    pt = ppt.tile([128, CL], F32, tag="tk")
                    nc.tensor.matmul(pt[:], src[:, ns * B * TV:(ns + 1) * B * TV],
                                     id16[:], start=True, stop=True)
                    nc.vector.tensor_copy(dst[:, ns * CL:(ns + 1) * CL], pt[:])

            # alpha/beta tiles [128, 32]
            alpha = pc.tile([128, NS * CL], F32, tag="alpha")
            beta = pc.tile([128, NS * CL], F32, tag="beta")
            for ns in range(NS):
                s = slice(ns * CL, (ns + 1) * CL)
                nc.vector.tensor_tensor(alpha[:, s], tatt[:, s],
                                        bcab[:, 0:16], OP.mult)
                nc.vector.tensor_tensor(beta[:, s], tatt[:, s],
                                        bcab[:, 16:32], OP.mult)

            # ---------------- fused output loop ---------------------------
            for c in range(CL):
                src = a1c(c)
                gate = pgate.tile([128, CF], F32, tag="gate")
                nc.scalar.activation(gate[:], src, AF.Relu,
                                     bias=bcab[:, 48 + c:49 + c],
                                     scale=bcab[:, 32 + c:33 + c])
                s0 = ps0.tile([128, CF], F32, tag="s0")
                nc.scalar.activation(s0[:], src, AF.Identity,
                                     bias=beta[:, c:c + 1],
                                     scale=alpha[:, c:c + 1])
                s1 = ps1.tile([128, CF], F32, tag="s1")
                if c % 2 == 0:
                    nc.scalar.activation(s1[:], src, AF.Identity,
                                         bias=beta[:, CL + c:CL + c + 1],
                                         scale=alpha[:, CL + c:CL + c + 1])
                else:
                    nc.vector.tensor_scalar(s1[:], src,
                                            alpha[:, CL + c:CL + c + 1],
                                            beta[:, CL + c:CL + c + 1],
                                            OP.mult, OP.add)
                ost = pout.tile([128, NS * CF], F32, tag="ost")
                nc.vector.scalar_tensor_tensor(
                    ost[:, 0:CF], gate[:], tkey[:, c:c + 1], s0[:],
                    OP.mult, OP.add)
                nc.vector.scalar_tensor_tensor(
                    ost[:, CF:2 * CF], gate[:], tkey[:, CL + c:CL + c + 1],
                    s1[:], OP.mult, OP.add)
                nc.sync.dma_start(
                    d_out.ap()[:, c * NS * CF:(c + 1) * NS * CF], ost[:])

    nc.compile()
    return nc


_NC_CACHE = None


def _get_nc():
    global _NC_CACHE
    if _NC_CACHE is None:
        _NC_CACHE = _build()
    return _NC_CACHE


def _pack_inputs(a1, v1, w_gate, b_gate, g_gate, be_gate,
                 w_val, b_val, g_val, be_val,
                 w_attn, b_attn, g_attn, be_attn,
                 w_key, b_key, g_key, be_key):
    f32 = np.float32
    a1 = np.asarray(a1, f32)
    v1 = np.asarray(v1, f32)
    # full-channel tensors (replicated)
    v1f = np.ascontiguousarray(v1.transpose(1, 0, 2).reshape(CA, N * TV))
    wa2 = np.asarray(w_attn, f32).reshape(CA, H)
    ba2 = np.asarray(b_attn, f32).reshape(CA, H)
    ga2 = np.asarray(g_attn, f32).reshape(CA, H)
    bea2 = np.asarray(be_attn, f32).reshape(CA, H)
    pcol = np.concatenate(
        [np.asarray(w_key, f32)[:, None], np.asarray(b_key, f32)[:, None],
         wa2, ba2], axis=1)
    pcol = np.ascontiguousarray(pcol)
    onec = np.ones((128, 1), f32)
    oner = np.ones((1, 128), f32)
    id16 = np.eye(CL, dtype=f32)

    in_maps = []
    for i in range(NCORE):
        sl = slice(i * CL, (i + 1) * CL)
        x = a1[:, sl].reshape(B, CL, TV, RP, FQ)
        x = np.ascontiguousarray(x.transpose(0, 2, 1, 3, 4))
        a1s = x.reshape(128, AFREE)
        v1l = np.ascontiguousarray(
            v1[:, sl].transpose(1, 0, 2).reshape(CL, N * TV))
        ga4 = ga2[sl] * 0.25
        ploc = np.concatenate(
            [np.asarray(w_key, f32)[sl, None], np.asarray(b_key, f32)[sl, None],
             np.asarray(g_key, f32)[sl, None], np.asarray(be_key, f32)[sl, None],
             wa2[sl], ba2[sl], ga4,
             ga4.sum(1, keepdims=True),
             (bea2[sl] * 0.25).sum(1, keepdims=True)], axis=1)
        ploc = np.ascontiguousarray(ploc)
        prow = np.concatenate(
            [np.asarray(w_val, f32)[sl], np.asarray(g_val, f32)[sl],
             np.asarray(be_val, f32)[sl], np.asarray(w_gate, f32)[sl],
             np.asarray(g_gate, f32)[sl],
             np.asarray(be_gate, f32)[sl]])[None, :]
        prow = np.ascontiguousarray(prow)
        in_maps.append({
            "a1s": a1s, "v1f": v1f, "v1l": v1l, "pcol": pcol,
            "ploc": ploc, "prow": prow, "onec": onec, "oner": oner,
            "id16": id16,
        })
    return in_maps


def _unpack_output(results):
    out = np.empty((N, CA, T, FQ), np.float32)
    for i in range(NCORE):
        r = np.asarray(results[i]["out"]).reshape(B, TV, CL, NS, RP, FQ)
        r = r.transpose(0, 3, 2, 1, 4, 5).reshape(N, CL, T, FQ)
        out[:, i * CL:(i + 1) * CL] = r
    return out


def _install_ntff_shim():
    """The agent image's ``antenv`` lacks ``axon_hooks``; recreate it and
    register the ctypes NTFF hook against /opt/axon/libaxon_pjrt.so (the
    same mechanism trn_boot uses when the module exists)."""
    import sys
    import types
    import ctypes
    import contextlib

    if "antenv.axon_hooks" in sys.modules:
        return True
    so_path = "/opt/axon/libaxon_pjrt.so"
    try:
        lib = ctypes.CDLL(so_path)
    except OSError:
        return False
    if not hasattr(lib, "axon_start_nrt_profile"):
        return False
    lib.axon_start_nrt_profile.argtypes = [ctypes.POINTER(ctypes.c_int64),
                                           ctypes.c_size_t]
    lib.axon_start_nrt_profile.restype = ctypes.c_int64
    lib.axon_stop_nrt_profile.argtypes = [ctypes.c_char_p]
    lib.axon_stop_nrt_profile.restype = ctypes.c_int64

    @contextlib.contextmanager
    def _hook(output_dir, device_ids):
        import jax
        jax.devices()
        if device_ids:
            ids = (ctypes.c_int64 * len(device_ids))(*device_ids)
            rc = lib.axon_start_nrt_profile(ids, len(device_ids))
        else:
            rc = lib.axon_start_nrt_profile(None, 0)
        if rc != 0:
            raise RuntimeError(f"axon_start_nrt_profile rc={rc}")
        try:
            yield
        finally:
            n = lib.axon_stop_nrt_profile(str(output_dir).encode())
            print(f"profile: {n} file(s) written to {output_dir}",
                  file=sys.stderr)

    mod = types.ModuleType("antenv.axon_hooks")
    _state = {"hook": _hook}
    mod.get_axon_ntff_profile_hook = lambda: _state["hook"]

    def set_axon_ntff_profile_hook(h):
        _state["hook"] = h

    mod.set_axon_ntff_profile_hook = set_axon_ntff_profile_hook
    import antenv
    antenv.axon_hooks = mod
    sys.modules["antenv.axon_hooks"] = mod
    return True


def run(inputs, trace=False, **trace_kwargs):
    """Returns (output, BassKernelResults)."""
    nc = _get_nc()
    in_maps = _pack_inputs(**inputs)
    if trace and not _install_ntff_shim():
        trace = False
    br = run_bass_kernel_spmd(nc, in_maps, core_ids=list(range(NCORE)),
                              trace=trace, **trace_kwargs)
    return _unpack_output(br.results), br


def kernel(**inputs):
    out, _ = run(inputs)
    return out


# revision 45
# speedup vs baseline: 1.0033x; 1.0033x over previous
"""Bass/Tile Trainium2 kernel for the CAFBlock fusion (nn_CAFBlock).

Strategy: shard the audio channel dim C_a=128 across 8 NeuronCores (16
channels per core).  BatchNorm2d statistics are per-channel -> fully local.
The tiny video branch (gLN over all channels) is computed redundantly on
every core from a replicated copy of v1, so there are no collectives.

Per-core SBUF layout for the big tensors: partition p = b*64 + k where k is
the video-frame index (t = k*8 + r), free dim = (c_local, r, f).  With this
layout the nearest-interpolated v_attn/v_key factors are constant along the
free dim, so each fused output tile needs only per-partition [128,1] scalar
operands:
    out[ns,c] = a1*(Av*attn) + (Bv*attn) + v_key * relu(a1*Ag + Bg)
computed as one ACT/DVE affine op + one scalar_tensor_tensor op.
"""

import numpy as np

import concourse.bass as bass
import concourse.bacc as bacc
import concourse.tile as tile
import concourse.mybir as mybir
from concourse.bass_utils import run_bass_kernel_spmd

F32 = mybir.dt.float32
AF = mybir.ActivationFunctionType
OP = mybir.AluOpType
AX = mybir.AxisListType
MS = bass.MemorySpace

# problem dims (hardcoded per the harness contract)
B, NS, CA, H, T, FQ, TV = 2, 2, 128, 4, 512, 128, 64
NCORE = 8
CL = CA // NCORE            # 16 local channels per core
N = B * NS                  # 4 (b*ns video samples)
RP = T // TV                # 8 (nearest-interp repeat factor)
BN_EPS, GLN_EPS = 1e-5, 1e-8
NBN = float(B * T * FQ)     # 131072 elements per BN channel
NKEY = float(CA * TV)       # 8192 elements per gLN(key) sample
NATT = float(CA * H * TV)   # 32768 elements per gLN(attn) sample
CF = RP * FQ                # 1024 free elements per channel tile
AFREE = CL * CF             # 16384 free elements of resident a1 shard
OFREE = CL * NS * CF        # 32768 free elements of output


def _build():
    """Builds the SPMD Bass program (same program on all 8 cores)."""
    nc = bacc.Bacc("TRN2", target_bir_lowering=False, debug=False)

    d_a1 = nc.dram_tensor("a1s", [128, AFREE], F32, kind="ExternalInput")
    # consts packed host-side into 3 tensors so they land in 3 fast DMAs
    # cb1 [128, 267]: v1f 0:256 | pcol 256:266 | onec 266:267
    # cb2 [16, 290]:  v1l 0:256 | ploc 256:274 | id16 274:290
    # cb3 [1, 224]:   oner 0:128 | prow 128:224
    d_cb1 = nc.dram_tensor("cb1", [128, 267], F32, kind="ExternalInput")
    d_cb2 = nc.dram_tensor("cb2", [CL, 290], F32, kind="ExternalInput")
    d_cb3 = nc.dram_tensor("cb3", [1, 224], F32, kind="ExternalInput")
    d_out = nc.dram_tensor("out", [128, OFREE], F32, kind="ExternalOutput")

    with tile.TileContext(nc) as tc:
        with (
            tc.tile_pool(name="pres", bufs=8) as pres,
            tc.tile_pool(name="pconst", bufs=1) as pc,
            tc.tile_pool(name="pscr", bufs=2) as pscr,
            tc.tile_pool(name="pgate", bufs=4) as pgate,
            tc.tile_pool(name="ps0", bufs=4) as ps0,
            tc.tile_pool(name="ps1", bufs=4) as ps1,
            tc.tile_pool(name="pout", bufs=3) as pout,
            tc.tile_pool(name="pps", bufs=1, space=MS.PSUM) as pps,
            tc.tile_pool(name="ppt", bufs=2, space=MS.PSUM) as ppt,
            tc.tile_pool(name="ppb", bufs=1, space=MS.PSUM) as ppb,
        ):
            # ---------------- constants first (3 fast HWDGE DMAs) ---------
            cb1 = pc.tile([128, 267], F32, tag="cb1")
            cb2 = pc.tile([CL, 290], F32, tag="cb2")
            cb3 = pc.tile([1, 224], F32, tag="cb3")
            nc.sync.dma_start(cb1[:], d_cb1.ap()[:])
            nc.sync.dma_start(cb2[:], d_cb2.ap()[:])
            nc.sync.dma_start(cb3[:], d_cb3.ap()[:])
            v1f = cb1[:, 0:256]
            pcol = cb1[:, 256:266]
            onec = cb1[:, 266:267]
            v1l = cb2[:, 0:256]
            ploc = cb2[:, 256:274]
            id16 = cb2[:, 274:290]
            oner = cb3[:, 0:128]
            prow = cb3[:, 128:224]

            # ---------------- input DMAs (HWDGE, 1MB each) ----------------
            res = []
            for g in range(8):
                t = pres.tile([128, 2048], F32, tag="res")
                nc.sync.dma_start(t[:], d_a1.ap()[:, g * 2048:(g + 1) * 2048])
                res.append(t)

            def a1c(c):
                return res[c // 2][:, (c % 2) * CF:(c % 2) * CF + CF]

            # ---------------- v-branch full-channel stats -----------------
            # key: vk = v1*wk + bk ; per-sample sums over (c, tv)
            vkf = pc.tile([128, N * TV], F32, tag="vkf")
            nc.vector.tensor_scalar(vkf[:], v1f[:], pcol[:, 0:1], pcol[:, 1:2],
                                    OP.mult, OP.add)
            ks = pc.tile([128, 8], F32, tag="ks")
            nc.vector.tensor_reduce(
                ks[:, 0:4], vkf[:].rearrange("p (n t) -> p n t", n=N, t=TV),
                axis=AX.X, op=OP.add)
            scrk = pc.tile([128, N * TV], F32, tag="scrk")
            nc.scalar.activation(scrk[:], vkf[:], AF.Square)
            nc.vector.tensor_reduce(
                ks[:, 4:8], scrk[:].rearrange("p (n t) -> p n t", n=N, t=TV),
                axis=AX.X, op=OP.add)

            # attn: va[h] = v1*wa_h + ba_h ; per-sample sums over (c, h, tv)
            va = pc.tile([128, H * N * TV], F32, tag="va")
            for h in range(H):
                nc.vector.tensor_scalar(
                    va[:, h * N * TV:(h + 1) * N * TV], v1f[:],
                    pcol[:, 2 + h:3 + h], pcol[:, 6 + h:7 + h], OP.mult, OP.add)
            asum = pc.tile([128, 8], F32, tag="asum")
            nc.vector.tensor_reduce(
                asum[:, 0:4],
                va[:].rearrange("p (h n t) -> p n h t", h=H, n=N, t=TV),
                axis=AX.XY, op=OP.add)
            scra = pc.tile([128, H * N * TV], F32, tag="scra")
            nc.scalar.activation(scra[:], va[:], AF.Square)
            nc.vector.tensor_reduce(
                asum[:, 4:8],
                scra[:].rearrange("p (h n t) -> p n h t", h=H, n=N, t=TV),
                axis=AX.XY, op=OP.add)

            # ------- v-branch cross-partition reduction + finalize --------
            # (independent of the BN stats: runs while a1 is still streaming)
            pp_ks = pps.tile([1, 8], F32, tag="ppks")
            pp_as = pps.tile([1, 8], F32, tag="ppas")
            nc.tensor.matmul(pp_ks[:], onec[:], ks[:], start=True, stop=True)
            nc.tensor.matmul(pp_as[:], onec[:], asum[:], start=True, stop=True)
            kr = pc.tile([1, 8], F32, tag="kr")
            ar = pc.tile([1, 8], F32, tag="ar")
            nc.scalar.copy(kr[:], pp_ks[:])
            nc.scalar.copy(ar[:], pp_as[:])

            # v rows: kv layout 0:4 kmean | 4:8 kex2 | 8:12 kvar | 12:16 km^2
            #         16:20 amean | 20:24 aex2 | 24:28 avar | 28:32 am^2
            kv = pc.tile([1, 32], F32, tag="kv")
            nc.vector.tensor_scalar_mul(kv[:, 0:4], kr[:, 0:4], 1.0 / NKEY)
            nc.vector.tensor_scalar_mul(kv[:, 4:8], kr[:, 4:8], 1.0 / NKEY)
            nc.vector.tensor_tensor(kv[:, 12:16], kv[:, 0:4], kv[:, 0:4],
                                    OP.mult)
            nc.vector.tensor_tensor(kv[:, 8:12], kv[:, 4:8], kv[:, 12:16],
                                    OP.subtract)
            nc.vector.tensor_scalar_mul(kv[:, 16:20], ar[:, 0:4], 1.0 / NATT)
            nc.vector.tensor_scalar_mul(kv[:, 20:24], ar[:, 4:8], 1.0 / NATT)
            nc.vector.tensor_tensor(kv[:, 28:32], kv[:, 16:20], kv[:, 16:20],
                                    OP.mult)
            nc.vector.tensor_tensor(kv[:, 24:28], kv[:, 20:24], kv[:, 28:32],
                                    OP.subtract)

            def rsqrt_rows(qa, width, pref):
                # 1/sqrt(q) via exp(-0.5*ln(q)) + one Newton polish
                lnq = pc.tile([1, width], F32, tag=pref + "ln")
                r0 = pc.tile([1, width], F32, tag=pref + "r0")
                rr = pc.tile([1, width], F32, tag=pref + "rr")
                ntt = pc.tile([1, width], F32, tag=pref + "nt")
                nc.scalar.activation(lnq[:], qa, AF.Ln)
                nc.scalar.activation(r0[:], lnq[:], AF.Exp, scale=-0.5)
                nc.vector.tensor_tensor(ntt[:], r0[:], r0[:], OP.mult)
                nc.vector.tensor_tensor(ntt[:], qa, ntt[:], OP.mult)
                nc.vector.tensor_scalar(ntt[:], ntt[:], -1.0, 3.0, OP.mult,
                                        OP.add)
                nc.vector.tensor_scalar_mul(rr[:], r0[:], 0.5)
                nc.vector.tensor_tensor(rr[:], rr[:], ntt[:], OP.mult)
                return rr

            qv = pc.tile([1, 8], F32, tag="qv")
            nc.vector.tensor_scalar_add(qv[:, 0:4], kv[:, 8:12], GLN_EPS)
            nc.vector.tensor_scalar_add(qv[:, 4:8], kv[:, 24:28], GLN_EPS)
            rsv = rsqrt_rows(qv[:], 8, "v")  # 0:4 rs_key | 4:8 rs_attn

            # bc1 row [1,28]: kmean(4) | rs_key(4) | amean*rs_attn(4) |
            #                 rs_attn repeated n-major h-minor (16)
            b1 = pc.tile([1, 28], F32, tag="b1")
            nc.vector.tensor_copy(b1[:, 0:4], kv[:, 0:4])
            nc.vector.tensor_copy(b1[:, 4:8], rsv[:, 0:4])
            nc.vector.tensor_tensor(b1[:, 8:12], kv[:, 16:20], rsv[:, 4:8],
                                    OP.mult)
            b1rep = b1[:, 12:28].rearrange("p (n x) -> p n x", n=N, x=H)
            rsat = rsv[:, 4:8].rearrange("p (n x) -> p n x", n=N, x=1)
            for h in range(H):
                nc.vector.tensor_copy(b1rep[:, :, h:h + 1], rsat[:])

            pp_b1 = ppb.tile([128, 28], F32, tag="ppb1")
            nc.tensor.matmul(pp_b1[:], oner[:], b1[:], start=True, stop=True)
            bc1 = pc.tile([128, 28], F32, tag="bc1")
            nc.scalar.copy(bc1[:], pp_b1[:])

            # ---------------- local v-branch ------------------------------
            # ploc: wk 0 | bk 1 | gk 2 | bek 3 | wa 4:8 | ba 8:12 |
            #       ga/4 12:16 | sum(ga)/4 16 | sum(bea)/4 17
            vkl = pc.tile([CL, N * TV], F32, tag="vkl")
            nc.vector.tensor_scalar(vkl[:], v1l[:], ploc[:, 0:1], ploc[:, 1:2],
                                    OP.mult, OP.add)
            kscol = pc.tile([CL, N], F32, tag="kscol")
            kbcol = pc.tile([CL, N], F32, tag="kbcol")
            for n in range(N):
                nc.vector.tensor_tensor(kscol[:, n:n + 1], ploc[:, 2:3],
                                        bc1[0:CL, 4 + n:5 + n], OP.mult)
                nc.vector.tensor_tensor(kbcol[:, n:n + 1], kscol[:, n:n + 1],
                                        bc1[0:CL, n:n + 1], OP.mult)
                nc.vector.tensor_tensor(kbcol[:, n:n + 1], ploc[:, 3:4],
                                        kbcol[:, n:n + 1], OP.subtract)
            # vkln/soft stored in (ns, b, tv) column order so the transpose
            # lhsT slice [16, 128] is contiguous: perm(n) = (n%2)*2 + n//2
            perm = [(n % 2) * 2 + n // 2 for n in range(N)]
            vkln = pc.tile([CL, N * TV], F32, tag="vkln")
            for n in range(N):
                nc.vector.tensor_scalar(
                    vkln[:, perm[n] * TV:(perm[n] + 1) * TV],
                    vkl[:, n * TV:(n + 1) * TV],
                    kscol[:, n:n + 1], kbcol[:, n:n + 1], OP.mult, OP.add)

            val = pc.tile([CL, H * N * TV], F32, tag="val")
            for h in range(H):
                nc.vector.tensor_scalar(
                    val[:, h * N * TV:(h + 1) * N * TV], v1l[:],
                    ploc[:, 4 + h:5 + h], ploc[:, 8 + h:9 + h], OP.mult, OP.add)
            ga16 = pc.tile([CL, N * H], F32, tag="ga16")
            for n in range(N):
                nc.vector.tensor_copy(ga16[:, n * H:(n + 1) * H],
                                      ploc[:, 12:16])
            sc16 = pc.tile([CL, N * H], F32, tag="sc16")
            nc.vector.tensor_tensor(sc16[:], ga16[:], bc1[0:CL, 12:28],
                                    OP.mult)
            bicol = pc.tile([CL, N], F32, tag="bicol")
            for n in range(N):
                nc.vector.tensor_tensor(bicol[:, n:n + 1], ploc[:, 16:17],
                                        bc1[0:CL, 8 + n:9 + n], OP.mult)
                nc.vector.tensor_tensor(bicol[:, n:n + 1], ploc[:, 17:18],
                                        bicol[:, n:n + 1], OP.subtract)
            # vm[n] = sum_h val[h,n]*sc16[n,h] + bicol[n]  (ga,bea host-/4)
            vm = pc.tile([CL, N * TV], F32, tag="vm")
            for n in range(N):
                dst = vm[:, n * TV:(n + 1) * TV]
                nc.vector.tensor_scalar(
                    dst, val[:, n * TV:n * TV + TV],
                    sc16[:, n * H:n * H + 1], bicol[:, n:n + 1],
                    OP.mult, OP.add)
                for h in range(1, H):
                    nc.vector.scalar_tensor_tensor(
                        dst, val[:, h * N * TV + n * TV:h * N * TV + n * TV + TV],
                        sc16[:, n * H + h:n * H + h + 1], dst, OP.mult, OP.add)
            # softmax over tv per (c, n)
            mx = pc.tile([CL, N], F32, tag="mx")
            nc.vector.tensor_reduce(
                mx[:], vm[:].rearrange("p (n t) -> p n t", n=N, t=TV),
                axis=AX.X, op=OP.max)
            nmx = pc.tile([CL, N], F32, tag="nmx")
            nc.vector.tensor_scalar_mul(nmx[:], mx[:], -1.0)
            ex = pc.tile([CL, N * TV], F32, tag="ex")
            ssum = pc.tile([CL, N], F32, tag="ssum")
            for n in range(N):
                nc.scalar.activation(
                    ex[:, n * TV:(n + 1) * TV], vm[:, n * TV:(n + 1) * TV],
                    AF.Exp, bias=nmx[:, n:n + 1],
                    accum_out=ssum[:, n:n + 1])
            rcp = pc.tile([CL, N], F32, tag="rcp")
            nc.vector.reciprocal(rcp[:], ssum[:])
            soft = pc.tile([CL, N * TV], F32, tag="soft")
            for n in range(N):
                nc.vector.tensor_scalar_mul(
                    soft[:, perm[n] * TV:(perm[n] + 1) * TV],
                    ex[:, n * TV:(n + 1) * TV], rcp[:, n:n + 1])

            # ---------------- transpose to (b,k) x (ns,c) -----------------
            # out[(b,tv), c] = src[c, (2b+ns)*TV + tv] via lhsT^T @ I16 with a
            # strided lhsT view gathering both b halves (M=128, K=16).
            tkey = pc.tile([128, NS * CL], F32, tag="tkey")
            tatt = pc.tile([128, NS * CL], F32, tag="tatt")
            for (src, dst) in ((vkln, tkey), (soft, tatt)):
                for ns in range(NS):
                    pt = ppt.tile([128, CL], F32, tag="tk")
                    nc.tensor.matmul(pt[:], src[:, ns * B * TV:(ns + 1) * B * TV],
                                     id16[:], start=True, stop=True)
                    nc.scalar.copy(dst[:, ns * CL:(ns + 1) * CL], pt[:])

            # ---------------- BN stats (pipelined with input DMA) ---------
            # per-partition sums on DVE (ts + accum_out); sums of squares on
            # ACT (Square + accum_out), which is otherwise idle here
            sums = pc.tile([128, CL], F32, tag="sums")
            sqs = pc.tile([128, CL], F32, tag="sqs")
            for c in range(CL):
                scrd = pscr.tile([128, CF], F32, tag="scrd")
                nc.vector.tensor_scalar(scrd[:], a1c(c), 1.0, None, OP.mult,
                                        OP.add, accum_out=sums[:, c:c + 1])
                scrs = pscr.tile([128, CF], F32, tag="scrs")
                nc.scalar.activation(scrs[:], a1c(c), AF.Square,
                                     accum_out=sqs[:, c:c + 1])

            pp_sm = pps.tile([1, CL], F32, tag="ppsm")
            pp_sq = pps.tile([1, CL], F32, tag="ppsq")
            nc.tensor.matmul(pp_sm[:], onec[:], sums[:], start=True, stop=True)
            nc.tensor.matmul(pp_sq[:], onec[:], sqs[:], start=True, stop=True)
            sm = pc.tile([1, CL], F32, tag="sm")
            sq = pc.tile([1, CL], F32, tag="sq")
            nc.scalar.copy(sm[:], pp_sm[:])
            nc.scalar.copy(sq[:], pp_sq[:])

            # rw layout: 0:16 mean | 16:32 ex2 | 32:48 mts | 48:64 var
            rw = pc.tile([1, 64], F32, tag="rw")
            nc.vector.tensor_scalar_mul(rw[:, 0:16], sm[:], 1.0 / NBN)
            nc.vector.tensor_scalar_mul(rw[:, 16:32], sq[:], 1.0 / NBN)
            nc.vector.tensor_tensor(rw[:, 32:48], rw[:, 0:16], rw[:, 0:16],
                                    OP.mult)
            nc.vector.tensor_tensor(rw[:, 48:64], rw[:, 16:32], rw[:, 32:48],
                                    OP.subtract)

            # qb [1,32]: var*wv^2+eps | var*wg^2+eps
            # prow layout: wv 0:16 | gv 16:32 | bev 32:48 | wg 48:64
            #              gg 64:80 | beg 80:96
            qb = pc.tile([1, 32], F32, tag="qb")
            w2 = pc.tile([1, 32], F32, tag="w2")
            nc.vector.tensor_tensor(w2[:, 0:16], prow[:, 0:16], prow[:, 0:16],
                                    OP.mult)
            nc.vector.tensor_tensor(w2[:, 16:32], prow[:, 48:64],
                                    prow[:, 48:64], OP.mult)
            nc.vector.tensor_tensor(qb[:, 0:16], rw[:, 48:64], w2[:, 0:16],
                                    OP.mult)
            nc.vector.tensor_tensor(qb[:, 16:32], rw[:, 48:64], w2[:, 16:32],
                                    OP.mult)
            nc.vector.tensor_scalar_add(qb[:], qb[:], BN_EPS)
            rsb = rsqrt_rows(qb[:], 32, "b")  # 0:16 val | 16:32 gate

            # Av/Bv/Ag/Bg row [1,64]
            ab = pc.tile([1, 64], F32, tag="ab")
            nc.vector.tensor_tensor(ab[:, 0:16], rsb[:, 0:16], prow[:, 16:32],
                                    OP.mult)
            nc.vector.tensor_tensor(ab[:, 0:16], ab[:, 0:16], prow[:, 0:16],
                                    OP.mult)
            nc.vector.tensor_tensor(ab[:, 16:32], rw[:, 0:16], ab[:, 0:16],
                                    OP.mult)
            nc.vector.tensor_tensor(ab[:, 16:32], prow[:, 32:48], ab[:, 16:32],
                                    OP.subtract)
            nc.vector.tensor_tensor(ab[:, 32:48], rsb[:, 16:32],
                                    prow[:, 64:80], OP.mult)
            nc.vector.tensor_tensor(ab[:, 32:48], ab[:, 32:48], prow[:, 48:64],
                                    OP.mult)
            nc.vector.tensor_tensor(ab[:, 48:64], rw[:, 0:16], ab[:, 32:48],
                                    OP.mult)
            nc.vector.tensor_tensor(ab[:, 48:64], prow[:, 80:96], ab[:, 48:64],
                                    OP.subtract)

            pp_ab = ppb.tile([128, 64], F32, tag="ppab")
            nc.tensor.matmul(pp_ab[:], oner[:], ab[:], start=True, stop=True)
            bcab = pc.tile([128, 64], F32, tag="bcab")
            nc.scalar.copy(bcab[:], pp_ab[:])

            # alpha/beta tiles [128, 32]
            alpha = pc.tile([128, NS * CL], F32, tag="alpha")
            beta = pc.tile([128, NS * CL], F32, tag="beta")
            for ns in range(NS):
                s = slice(ns * CL, (ns + 1) * CL)
                nc.vector.tensor_tensor(alpha[:, s], tatt[:, s],
                                        bcab[:, 0:16], OP.mult)
                nc.vector.tensor_tensor(beta[:, s], tatt[:, s],
                                        bcab[:, 16:32], OP.mult)

            # ---------------- fused output loop ---------------------------
            for c in range(CL):
                src = a1c(c)
                gate = pgate.tile([128, CF], F32, tag="gate")
                nc.scalar.activation(gate[:], src, AF.Relu,
                                     bias=bcab[:, 48 + c:49 + c],
                                     scale=bcab[:, 32 + c:33 + c])
                s0 = ps0.tile([128, CF], F32, tag="s0")
                nc.scalar.activation(s0[:], src, AF.Identity,
                                     bias=beta[:, c:c + 1],
                                     scale=alpha[:, c:c + 1])
                s1 = ps1.tile([128, CF], F32, tag="s1")
                if c % 4 != 3:
                    nc.vector.tensor_scalar(s1[:], src,
                                            alpha[:, CL + c:CL + c + 1],
                                            beta[:, CL + c:CL + c + 1],
                                            OP.mult, OP.add)
                else:
                    nc.scalar.activation(s1[:], src, AF.Identity,
                                         bias=beta[:, CL + c:CL + c + 1],
                                         scale=alpha[:, CL + c:CL + c + 1])
                if c % 2 == 0:
                    ost = pout.tile([128, 2 * NS * CF], F32, tag="ost")
                base = (c % 2) * NS * CF
                nc.vector.scalar_tensor_tensor(
                    ost[:, base:base + CF], gate[:], tkey[:, c:c + 1], s0[:],
                    OP.mult, OP.add)
                nc.vector.scalar_tensor_tensor(
                    ost[:, base + CF:base + 2 * CF], gate[:],
                    tkey[:, CL + c:CL + c + 1], s1[:], OP.mult, OP.add)
                if c % 2 == 1:
                    nc.sync.dma_start(
                        d_out.ap()[:, (c - 1) * NS * CF:(c + 1) * NS * CF],
                        ost[:])

    nc.compile()
    return nc


_NC_CACHE = None


def _get_nc():
    global _NC_CACHE
    if _NC_CACHE is None:
        _NC_CACHE = _build()
    return _NC_CACHE


def _pack_inputs(a1, v1, w_gate, b_gate, g_gate, be_gate,
                 w_val, b_val, g_val, be_val,
                 w_attn, b_attn, g_attn, be_attn,
                 w_key, b_key, g_key, be_key):
    f32 = np.float32
    a1 = np.asarray(a1, f32)
    v1 = np.asarray(v1, f32)
    # full-channel tensors (replicated)
    v1f = np.ascontiguousarray(v1.transpose(1, 0, 2).reshape(CA, N * TV))
    wa2 = np.asarray(w_attn, f32).reshape(CA, H)
    ba2 = np.asarray(b_attn, f32).reshape(CA, H)
    ga2 = np.asarray(g_attn, f32).reshape(CA, H)
    bea2 = np.asarray(be_attn, f32).reshape(CA, H)
    pcol = np.concatenate(
        [np.asarray(w_key, f32)[:, None], np.asarray(b_key, f32)[:, None],
         wa2, ba2], axis=1)
    cb1 = np.concatenate([v1f, pcol, np.ones((CA, 1), f32)], axis=1)
    cb1 = np.ascontiguousarray(cb1)
    id16 = np.eye(CL, dtype=f32)

    in_maps = []
    for i in range(NCORE):
        sl = slice(i * CL, (i + 1) * CL)
        x = a1[:, sl].reshape(B, CL, TV, RP, FQ)
        x = np.ascontiguousarray(x.transpose(0, 2, 1, 3, 4))
        a1s = x.reshape(128, AFREE)
        v1l = np.ascontiguousarray(
            v1[:, sl].transpose(1, 0, 2).reshape(CL, N * TV))
        ga4 = ga2[sl] * 0.25
        ploc = np.concatenate(
            [np.asarray(w_key, f32)[sl, None], np.asarray(b_key, f32)[sl, None],
             np.asarray(g_key, f32)[sl, None], np.asarray(be_key, f32)[sl, None],
             wa2[sl], ba2[sl], ga4,
             ga4.sum(1, keepdims=True),
             (bea2[sl] * 0.25).sum(1, keepdims=True)], axis=1)
        cb2 = np.ascontiguousarray(
            np.concatenate([v1l, ploc, id16], axis=1))
        prow = np.concatenate(
            [np.asarray(w_val, f32)[sl], np.asarray(g_val, f32)[sl],
             np.asarray(be_val, f32)[sl], np.asarray(w_gate, f32)[sl],
             np.asarray(g_gate, f32)[sl],
             np.asarray(be_gate, f32)[sl]])[None, :]
        cb3 = np.ascontiguousarray(
            np.concatenate([np.ones((1, 128), f32), prow], axis=1))
        in_maps.append({"a1s": a1s, "cb1": cb1, "cb2": cb2, "cb3": cb3})
    return in_maps


def _unpack_output(results):
    out = np.empty((N, CA, T, FQ), np.float32)
    for i in range(NCORE):
        r = np.asarray(results[i]["out"]).reshape(B, TV, CL, NS, RP, FQ)
        r = r.transpose(0, 3, 2, 1, 4, 5).reshape(N, CL, T, FQ)
        out[:, i * CL:(i + 1) * CL] = r
    return out


def _install_ntff_shim():
    """The agent image's ``antenv`` lacks ``axon_hooks``; recreate it and
    register the ctypes NTFF hook against /opt/axon/libaxon_pjrt.so (the
    same mechanism trn_boot uses when the module exists)."""
    import sys
    import types
    import ctypes
    import contextlib

    if "antenv.axon_hooks" in sys.modules:
        return True
    so_path = "/opt/axon/libaxon_pjrt.so"
    try:
        lib = ctypes.CDLL(so_path)
    except OSError:
        return False
    if not hasattr(lib, "axon_start_nrt_profile"):
        return False
    lib.axon_start_nrt_profile.argtypes = [ctypes.POINTER(ctypes.c_int64),
                                           ctypes.c_size_t]
    lib.axon_start_nrt_profile.restype = ctypes.c_int64
    lib.axon_stop_nrt_profile.argtypes = [ctypes.c_char_p]
    lib.axon_stop_nrt_profile.restype = ctypes.c_int64

    @contextlib.contextmanager
    def _hook(output_dir, device_ids):
        import jax
        jax.devices()
        if device_ids:
            ids = (ctypes.c_int64 * len(device_ids))(*device_ids)
            rc = lib.axon_start_nrt_profile(ids, len(device_ids))
        else:
            rc = lib.axon_start_nrt_profile(None, 0)
        if rc != 0:
            raise RuntimeError(f"axon_start_nrt_profile rc={rc}")
        try:
            yield
        finally:
            n = lib.axon_stop_nrt_profile(str(output_dir).encode())
            print(f"profile: {n} file(s) written to {output_dir}",
                  file=sys.stderr)

    mod = types.ModuleType("antenv.axon_hooks")
    _state = {"hook": _hook}
    mod.get_axon_ntff_profile_hook = lambda: _state["hook"]

    def set_axon_ntff_profile_hook(h):
        _state["hook"] = h

    mod.set_axon_ntff_profile_hook = set_axon_ntff_profile_hook
    import antenv
    antenv.axon_hooks = mod
    sys.modules["antenv.axon_hooks"] = mod
    return True


def run(inputs, trace=False, **trace_kwargs):
    """Returns (output, BassKernelResults)."""
    nc = _get_nc()
    in_maps = _pack_inputs(**inputs)
    if trace and not _install_ntff_shim():
        trace = False
    br = run_bass_kernel_spmd(nc, in_maps, core_ids=list(range(NCORE)),
                              trace=trace, **trace_kwargs)
    return _unpack_output(br.results), br


def kernel(**inputs):
    out, _ = run(inputs)
    return out


# revision 46
# speedup vs baseline: 1.0061x; 1.0029x over previous
"""Bass/Tile Trainium2 kernel for the CAFBlock fusion (nn_CAFBlock).

Strategy: shard the audio channel dim C_a=128 across 8 NeuronCores (16
channels per core).  BatchNorm2d statistics are per-channel -> fully local.
The tiny video branch (gLN over all channels) is computed redundantly on
every core from a replicated copy of v1, so there are no collectives.

Per-core SBUF layout for the big tensors: partition p = b*64 + k where k is
the video-frame index (t = k*8 + r), free dim = (c_local, r, f).  With this
layout the nearest-interpolated v_attn/v_key factors are constant along the
free dim, so each fused output tile needs only per-partition [128,1] scalar
operands:
    out[ns,c] = a1*(Av*attn) + (Bv*attn) + v_key * relu(a1*Ag + Bg)
computed as one ACT/DVE affine op + one scalar_tensor_tensor op.
"""

import numpy as np

import concourse.bass as bass
import concourse.bacc as bacc
import concourse.tile as tile
import concourse.mybir as mybir
from concourse.bass_utils import run_bass_kernel_spmd

F32 = mybir.dt.float32
AF = mybir.ActivationFunctionType
OP = mybir.AluOpType
AX = mybir.AxisListType
MS = bass.MemorySpace

# problem dims (hardcoded per the harness contract)
B, NS, CA, H, T, FQ, TV = 2, 2, 128, 4, 512, 128, 64
NCORE = 8
CL = CA // NCORE            # 16 local channels per core
N = B * NS                  # 4 (b*ns video samples)
RP = T // TV                # 8 (nearest-interp repeat factor)
BN_EPS, GLN_EPS = 1e-5, 1e-8
NBN = float(B * T * FQ)     # 131072 elements per BN channel
NKEY = float(CA * TV)       # 8192 elements per gLN(key) sample
NATT = float(CA * H * TV)   # 32768 elements per gLN(attn) sample
CF = RP * FQ                # 1024 free elements per channel tile
AFREE = CL * CF             # 16384 free elements of resident a1 shard
OFREE = CL * NS * CF        # 32768 free elements of output


def _build():
    """Builds the SPMD Bass program (same program on all 8 cores)."""
    nc = bacc.Bacc("TRN2", target_bir_lowering=False, debug=False)

    d_a1 = nc.dram_tensor("a1s", [128, AFREE], F32, kind="ExternalInput")
    # consts packed host-side into 3 tensors so they land in 3 fast DMAs
    # cb1 [128, 267]: v1f 0:256 | pcol 256:266 | onec 266:267
    # cb2 [16, 290]:  v1l 0:256 | ploc 256:274 | id16 274:290
    # cb3 [1, 224]:   oner 0:128 | prow 128:224
    d_cb1 = nc.dram_tensor("cb1", [128, 267], F32, kind="ExternalInput")
    d_cb2 = nc.dram_tensor("cb2", [CL, 290], F32, kind="ExternalInput")
    d_cb3 = nc.dram_tensor("cb3", [1, 224], F32, kind="ExternalInput")
    d_out = nc.dram_tensor("out", [128, OFREE], F32, kind="ExternalOutput")

    with tile.TileContext(nc) as tc:
        with (
            tc.tile_pool(name="pres", bufs=8) as pres,
            tc.tile_pool(name="pconst", bufs=1) as pc,
            tc.tile_pool(name="pscr", bufs=2) as pscr,
            tc.tile_pool(name="pgate", bufs=4) as pgate,
            tc.tile_pool(name="ps0", bufs=4) as ps0,
            tc.tile_pool(name="ps1", bufs=4) as ps1,
            tc.tile_pool(name="pout", bufs=3) as pout,
            tc.tile_pool(name="pps", bufs=1, space=MS.PSUM) as pps,
            tc.tile_pool(name="ppt", bufs=2, space=MS.PSUM) as ppt,
            tc.tile_pool(name="ppb", bufs=1, space=MS.PSUM) as ppb,
        ):
            # ---------------- constants first (3 fast HWDGE DMAs) ---------
            cb1 = pc.tile([128, 267], F32, tag="cb1")
            cb2 = pc.tile([CL, 290], F32, tag="cb2")
            cb3 = pc.tile([1, 224], F32, tag="cb3")
            nc.sync.dma_start(cb1[:], d_cb1.ap()[:])
            nc.sync.dma_start(cb2[:], d_cb2.ap()[:])
            nc.sync.dma_start(cb3[:], d_cb3.ap()[:])
            v1f = cb1[:, 0:256]
            pcol = cb1[:, 256:266]
            onec = cb1[:, 266:267]
            v1l = cb2[:, 0:256]
            ploc = cb2[:, 256:274]
            id16 = cb2[:, 274:290]
            oner = cb3[:, 0:128]
            prow = cb3[:, 128:224]

            # ---------------- input DMAs (HWDGE, 1MB each) ----------------
            res = []
            for g in range(8):
                t = pres.tile([128, 2048], F32, tag="res")
                nc.sync.dma_start(t[:], d_a1.ap()[:, g * 2048:(g + 1) * 2048])
                res.append(t)

            def a1c(c):
                return res[c // 2][:, (c % 2) * CF:(c % 2) * CF + CF]

            # ---------------- v-branch full-channel stats -----------------
            # key: vk = v1*wk + bk ; per-sample sums over (c, tv)
            vkf = pc.tile([128, N * TV], F32, tag="vkf")
            nc.vector.tensor_scalar(vkf[:], v1f[:], pcol[:, 0:1], pcol[:, 1:2],
                                    OP.mult, OP.add)
            ks = pc.tile([128, 8], F32, tag="ks")
            nc.vector.tensor_reduce(
                ks[:, 0:4], vkf[:].rearrange("p (n t) -> p n t", n=N, t=TV),
                axis=AX.X, op=OP.add)
            scrk = pc.tile([128, N * TV], F32, tag="scrk")
            nc.vector.tensor_tensor(scrk[:], vkf[:], vkf[:], OP.mult)
            nc.vector.tensor_reduce(
                ks[:, 4:8], scrk[:].rearrange("p (n t) -> p n t", n=N, t=TV),
                axis=AX.X, op=OP.add)

            # attn: va[h] = v1*wa_h + ba_h ; per-sample sums over (c, h, tv)
            va = pc.tile([128, H * N * TV], F32, tag="va")
            for h in range(H):
                nc.vector.tensor_scalar(
                    va[:, h * N * TV:(h + 1) * N * TV], v1f[:],
                    pcol[:, 2 + h:3 + h], pcol[:, 6 + h:7 + h], OP.mult, OP.add)
            asum = pc.tile([128, 8], F32, tag="asum")
            nc.vector.tensor_reduce(
                asum[:, 0:4],
                va[:].rearrange("p (h n t) -> p n h t", h=H, n=N, t=TV),
                axis=AX.XY, op=OP.add)
            scra = pc.tile([128, H * N * TV], F32, tag="scra")
            nc.vector.tensor_tensor(scra[:], va[:], va[:], OP.mult)
            nc.vector.tensor_reduce(
                asum[:, 4:8],
                scra[:].rearrange("p (h n t) -> p n h t", h=H, n=N, t=TV),
                axis=AX.XY, op=OP.add)

            # ------- v-branch cross-partition reduction + finalize --------
            # (independent of the BN stats: runs while a1 is still streaming)
            pp_ks = pps.tile([1, 8], F32, tag="ppks")
            pp_as = pps.tile([1, 8], F32, tag="ppas")
            nc.tensor.matmul(pp_ks[:], onec[:], ks[:], start=True, stop=True)
            nc.tensor.matmul(pp_as[:], onec[:], asum[:], start=True, stop=True)
            kr = pc.tile([1, 8], F32, tag="kr")
            ar = pc.tile([1, 8], F32, tag="ar")
            nc.scalar.copy(kr[:], pp_ks[:])
            nc.scalar.copy(ar[:], pp_as[:])

            # v rows: kv layout 0:4 kmean | 4:8 kex2 | 8:12 kvar | 12:16 km^2
            #         16:20 amean | 20:24 aex2 | 24:28 avar | 28:32 am^2
            kv = pc.tile([1, 32], F32, tag="kv")
            nc.vector.tensor_scalar_mul(kv[:, 0:4], kr[:, 0:4], 1.0 / NKEY)
            nc.vector.tensor_scalar_mul(kv[:, 4:8], kr[:, 4:8], 1.0 / NKEY)
            nc.vector.tensor_tensor(kv[:, 12:16], kv[:, 0:4], kv[:, 0:4],
                                    OP.mult)
            nc.vector.tensor_tensor(kv[:, 8:12], kv[:, 4:8], kv[:, 12:16],
                                    OP.subtract)
            nc.vector.tensor_scalar_mul(kv[:, 16:20], ar[:, 0:4], 1.0 / NATT)
            nc.vector.tensor_scalar_mul(kv[:, 20:24], ar[:, 4:8], 1.0 / NATT)
            nc.vector.tensor_tensor(kv[:, 28:32], kv[:, 16:20], kv[:, 16:20],
                                    OP.mult)
            nc.vector.tensor_tensor(kv[:, 24:28], kv[:, 20:24], kv[:, 28:32],
                                    OP.subtract)

            def rsqrt_rows(qa, width, pref):
                # 1/sqrt(q) via exp(-0.5*ln(q)) + one Newton polish
                lnq = pc.tile([1, width], F32, tag=pref + "ln")
                r0 = pc.tile([1, width], F32, tag=pref + "r0")
                rr = pc.tile([1, width], F32, tag=pref + "rr")
                ntt = pc.tile([1, width], F32, tag=pref + "nt")
                nc.scalar.activation(lnq[:], qa, AF.Ln)
                nc.scalar.activation(r0[:], lnq[:], AF.Exp, scale=-0.5)
                nc.vector.tensor_tensor(ntt[:], r0[:], r0[:], OP.mult)
                nc.vector.tensor_tensor(ntt[:], qa, ntt[:], OP.mult)
                nc.vector.tensor_scalar(ntt[:], ntt[:], -1.0, 3.0, OP.mult,
                                        OP.add)
                nc.vector.tensor_scalar_mul(rr[:], r0[:], 0.5)
                nc.vector.tensor_tensor(rr[:], rr[:], ntt[:], OP.mult)
                return rr

            qv = pc.tile([1, 8], F32, tag="qv")
            nc.vector.tensor_scalar_add(qv[:, 0:4], kv[:, 8:12], GLN_EPS)
            nc.vector.tensor_scalar_add(qv[:, 4:8], kv[:, 24:28], GLN_EPS)
            rsv = rsqrt_rows(qv[:], 8, "v")  # 0:4 rs_key | 4:8 rs_attn

            # bc1 row [1,28]: kmean(4) | rs_key(4) | amean*rs_attn(4) |
            #                 rs_attn repeated n-major h-minor (16)
            b1 = pc.tile([1, 28], F32, tag="b1")
            nc.vector.tensor_copy(b1[:, 0:4], kv[:, 0:4])
            nc.vector.tensor_copy(b1[:, 4:8], rsv[:, 0:4])
            nc.vector.tensor_tensor(b1[:, 8:12], kv[:, 16:20], rsv[:, 4:8],
                                    OP.mult)
            b1rep = b1[:, 12:28].rearrange("p (n x) -> p n x", n=N, x=H)
            rsat = rsv[:, 4:8].rearrange("p (n x) -> p n x", n=N, x=1)
            for h in range(H):
                nc.vector.tensor_copy(b1rep[:, :, h:h + 1], rsat[:])

            pp_b1 = ppb.tile([128, 28], F32, tag="ppb1")
            nc.tensor.matmul(pp_b1[:], oner[:], b1[:], start=True, stop=True)
            bc1 = pc.tile([128, 28], F32, tag="bc1")
            nc.scalar.copy(bc1[:], pp_b1[:])

            # ---------------- local v-branch ------------------------------
            # ploc: wk 0 | bk 1 | gk 2 | bek 3 | wa 4:8 | ba 8:12 |
            #       ga/4 12:16 | sum(ga)/4 16 | sum(bea)/4 17
            vkl = pc.tile([CL, N * TV], F32, tag="vkl")
            nc.vector.tensor_scalar(vkl[:], v1l[:], ploc[:, 0:1], ploc[:, 1:2],
                                    OP.mult, OP.add)
            kscol = pc.tile([CL, N], F32, tag="kscol")
            kbcol = pc.tile([CL, N], F32, tag="kbcol")
            for n in range(N):
                nc.vector.tensor_tensor(kscol[:, n:n + 1], ploc[:, 2:3],
                                        bc1[0:CL, 4 + n:5 + n], OP.mult)
                nc.vector.tensor_tensor(kbcol[:, n:n + 1], kscol[:, n:n + 1],
                                        bc1[0:CL, n:n + 1], OP.mult)
                nc.vector.tensor_tensor(kbcol[:, n:n + 1], ploc[:, 3:4],
                                        kbcol[:, n:n + 1], OP.subtract)
            # vkln/soft stored in (ns, b, tv) column order so the transpose
            # lhsT slice [16, 128] is contiguous: perm(n) = (n%2)*2 + n//2
            perm = [(n % 2) * 2 + n // 2 for n in range(N)]
            vkln = pc.tile([CL, N * TV], F32, tag="vkln")
            for n in range(N):
                nc.vector.tensor_scalar(
                    vkln[:, perm[n] * TV:(perm[n] + 1) * TV],
                    vkl[:, n * TV:(n + 1) * TV],
                    kscol[:, n:n + 1], kbcol[:, n:n + 1], OP.mult, OP.add)

            val = pc.tile([CL, H * N * TV], F32, tag="val")
            for h in range(H):
                nc.vector.tensor_scalar(
                    val[:, h * N * TV:(h + 1) * N * TV], v1l[:],
                    ploc[:, 4 + h:5 + h], ploc[:, 8 + h:9 + h], OP.mult, OP.add)
            ga16 = pc.tile([CL, N * H], F32, tag="ga16")
            for n in range(N):
                nc.vector.tensor_copy(ga16[:, n * H:(n + 1) * H],
                                      ploc[:, 12:16])
            sc16 = pc.tile([CL, N * H], F32, tag="sc16")
            nc.vector.tensor_tensor(sc16[:], ga16[:], bc1[0:CL, 12:28],
                                    OP.mult)
            bicol = pc.tile([CL, N], F32, tag="bicol")
            for n in range(N):
                nc.vector.tensor_tensor(bicol[:, n:n + 1], ploc[:, 16:17],
                                        bc1[0:CL, 8 + n:9 + n], OP.mult)
                nc.vector.tensor_tensor(bicol[:, n:n + 1], ploc[:, 17:18],
                                        bicol[:, n:n + 1], OP.subtract)
            # vm[n] = sum_h val[h,n]*sc16[n,h] + bicol[n]  (ga,bea host-/4)
            vm = pc.tile([CL, N * TV], F32, tag="vm")
            for n in range(N):
                dst = vm[:, n * TV:(n + 1) * TV]
                nc.vector.tensor_scalar(
                    dst, val[:, n * TV:n * TV + TV],
                    sc16[:, n * H:n * H + 1], bicol[:, n:n + 1],
                    OP.mult, OP.add)
                for h in range(1, H):
                    nc.vector.scalar_tensor_tensor(
                        dst, val[:, h * N * TV + n * TV:h * N * TV + n * TV + TV],
                        sc16[:, n * H + h:n * H + h + 1], dst, OP.mult, OP.add)
            # softmax over tv per (c, n)
            mx = pc.tile([CL, N], F32, tag="mx")
            nc.vector.tensor_reduce(
                mx[:], vm[:].rearrange("p (n t) -> p n t", n=N, t=TV),
                axis=AX.X, op=OP.max)
            nmx = pc.tile([CL, N], F32, tag="nmx")
            nc.vector.tensor_scalar_mul(nmx[:], mx[:], -1.0)
            ex = pc.tile([CL, N * TV], F32, tag="ex")
            ssum = pc.tile([CL, N], F32, tag="ssum")
            for n in range(N):
                nc.scalar.activation(
                    ex[:, n * TV:(n + 1) * TV], vm[:, n * TV:(n + 1) * TV],
                    AF.Exp, bias=nmx[:, n:n + 1],
                    accum_out=ssum[:, n:n + 1])
            rcp = pc.tile([CL, N], F32, tag="rcp")
            nc.vector.reciprocal(rcp[:], ssum[:])
            soft = pc.tile([CL, N * TV], F32, tag="soft")
            for n in range(N):
                nc.vector.tensor_scalar_mul(
                    soft[:, perm[n] * TV:(perm[n] + 1) * TV],
                    ex[:, n * TV:(n + 1) * TV], rcp[:, n:n + 1])

            # ---------------- transpose to (b,k) x (ns,c) -----------------
            # out[(b,tv), c] = src[c, (2b+ns)*TV + tv] via lhsT^T @ I16 with a
            # strided lhsT view gathering both b halves (M=128, K=16).
            tkey = pc.tile([128, NS * CL], F32, tag="tkey")
            tatt = pc.tile([128, NS * CL], F32, tag="tatt")
            for (src, dst) in ((vkln, tkey), (soft, tatt)):
                for ns in range(NS):
                    pt = ppt.tile([128, CL], F32, tag="tk")
                    nc.tensor.matmul(pt[:], src[:, ns * B * TV:(ns + 1) * B * TV],
                                     id16[:], start=True, stop=True)
                    nc.scalar.copy(dst[:, ns * CL:(ns + 1) * CL], pt[:])

            # ---------------- BN stats (pipelined with input DMA) ---------
            # per-partition sums on DVE (ts + accum_out); sums of squares on
            # ACT (Square + accum_out), which is otherwise idle here
            sums = pc.tile([128, CL], F32, tag="sums")
            sqs = pc.tile([128, CL], F32, tag="sqs")
            for c in range(CL):
                scrd = pscr.tile([128, CF], F32, tag="scrd")
                nc.vector.tensor_scalar(scrd[:], a1c(c), 1.0, None, OP.mult,
                                        OP.add, accum_out=sums[:, c:c + 1])
                scrs = pscr.tile([128, CF], F32, tag="scrs")
                nc.scalar.activation(scrs[:], a1c(c), AF.Square,
                                     accum_out=sqs[:, c:c + 1])

            pp_sm = pps.tile([1, CL], F32, tag="ppsm")
            pp_sq = pps.tile([1, CL], F32, tag="ppsq")
            nc.tensor.matmul(pp_sm[:], onec[:], sums[:], start=True, stop=True)
            nc.tensor.matmul(pp_sq[:], onec[:], sqs[:], start=True, stop=True)
            sm = pc.tile([1, CL], F32, tag="sm")
            sq = pc.tile([1, CL], F32, tag="sq")
            nc.scalar.copy(sm[:], pp_sm[:])
            nc.scalar.copy(sq[:], pp_sq[:])

            # rw layout: 0:16 mean | 16:32 ex2 | 32:48 mts | 48:64 var
            rw = pc.tile([1, 64], F32, tag="rw")
            nc.vector.tensor_scalar_mul(rw[:, 0:16], sm[:], 1.0 / NBN)
            nc.vector.tensor_scalar_mul(rw[:, 16:32], sq[:], 1.0 / NBN)
            nc.vector.tensor_tensor(rw[:, 32:48], rw[:, 0:16], rw[:, 0:16],
                                    OP.mult)
            nc.vector.tensor_tensor(rw[:, 48:64], rw[:, 16:32], rw[:, 32:48],
                                    OP.subtract)

            # qb [1,32]: var*wv^2+eps | var*wg^2+eps
            # prow layout: wv 0:16 | gv 16:32 | bev 32:48 | wg 48:64
            #              gg 64:80 | beg 80:96
            qb = pc.tile([1, 32], F32, tag="qb")
            w2 = pc.tile([1, 32], F32, tag="w2")
            nc.vector.tensor_tensor(w2[:, 0:16], prow[:, 0:16], prow[:, 0:16],
                                    OP.mult)
            nc.vector.tensor_tensor(w2[:, 16:32], prow[:, 48:64],
                                    prow[:, 48:64], OP.mult)
            nc.vector.tensor_tensor(qb[:, 0:16], rw[:, 48:64], w2[:, 0:16],
                                    OP.mult)
            nc.vector.tensor_tensor(qb[:, 16:32], rw[:, 48:64], w2[:, 16:32],
                                    OP.mult)
            nc.vector.tensor_scalar_add(qb[:], qb[:], BN_EPS)
            rsb = rsqrt_rows(qb[:], 32, "b")  # 0:16 val | 16:32 gate

            # Av/Bv/Ag/Bg row [1,64]
            ab = pc.tile([1, 64], F32, tag="ab")
            nc.vector.tensor_tensor(ab[:, 0:16], rsb[:, 0:16], prow[:, 16:32],
                                    OP.mult)
            nc.vector.tensor_tensor(ab[:, 0:16], ab[:, 0:16], prow[:, 0:16],
                                    OP.mult)
            nc.vector.tensor_tensor(ab[:, 16:32], rw[:, 0:16], ab[:, 0:16],
                                    OP.mult)
            nc.vector.tensor_tensor(ab[:, 16:32], prow[:, 32:48], ab[:, 16:32],
                                    OP.subtract)
            nc.vector.tensor_tensor(ab[:, 32:48], rsb[:, 16:32],
                                    prow[:, 64:80], OP.mult)
            nc.vector.tensor_tensor(ab[:, 32:48], ab[:, 32:48], prow[:, 48:64],
                                    OP.mult)
            nc.vector.tensor_tensor(ab[:, 48:64], rw[:, 0:16], ab[:, 32:48],
                                    OP.mult)
            nc.vector.tensor_tensor(ab[:, 48:64], prow[:, 80:96], ab[:, 48:64],
                                    OP.subtract)

            pp_ab = ppb.tile([128, 64], F32, tag="ppab")
            nc.tensor.matmul(pp_ab[:], oner[:], ab[:], start=True, stop=True)
            bcab = pc.tile([128, 64], F32, tag="bcab")
            nc.scalar.copy(bcab[:], pp_ab[:])

            # alpha/beta tiles [128, 32]
            alpha = pc.tile([128, NS * CL], F32, tag="alpha")
            beta = pc.tile([128, NS * CL], F32, tag="beta")
            for ns in range(NS):
                s = slice(ns * CL, (ns + 1) * CL)
                nc.vector.tensor_tensor(alpha[:, s], tatt[:, s],
                                        bcab[:, 0:16], OP.mult)
                nc.vector.tensor_tensor(beta[:, s], tatt[:, s],
                                        bcab[:, 16:32], OP.mult)

            # ---------------- fused output loop ---------------------------
            for c in range(CL):
                src = a1c(c)
                gate = pgate.tile([128, CF], F32, tag="gate")
                nc.scalar.activation(gate[:], src, AF.Relu,
                                     bias=bcab[:, 48 + c:49 + c],
                                     scale=bcab[:, 32 + c:33 + c])
                s0 = ps0.tile([128, CF], F32, tag="s0")
                nc.scalar.activation(s0[:], src, AF.Identity,
                                     bias=beta[:, c:c + 1],
                                     scale=alpha[:, c:c + 1])
                s1 = ps1.tile([128, CF], F32, tag="s1")
                if c % 4 != 3:
                    nc.vector.tensor_scalar(s1[:], src,
                                            alpha[:, CL + c:CL + c + 1],
                                            beta[:, CL + c:CL + c + 1],
                                            OP.mult, OP.add)
                else:
                    nc.scalar.activation(s1[:], src, AF.Identity,
                                         bias=beta[:, CL + c:CL + c + 1],
                                         scale=alpha[:, CL + c:CL + c + 1])
                if c % 2 == 0:
                    ost = pout.tile([128, 2 * NS * CF], F32, tag="ost")
                base = (c % 2) * NS * CF
                nc.vector.scalar_tensor_tensor(
                    ost[:, base:base + CF], gate[:], tkey[:, c:c + 1], s0[:],
                    OP.mult, OP.add)
                nc.vector.scalar_tensor_tensor(
                    ost[:, base + CF:base + 2 * CF], gate[:],
                    tkey[:, CL + c:CL + c + 1], s1[:], OP.mult, OP.add)
                if c % 2 == 1:
                    nc.sync.dma_start(
                        d_out.ap()[:, (c - 1) * NS * CF:(c + 1) * NS * CF],
                        ost[:])

    nc.compile()
    return nc


_NC_CACHE = None


def _get_nc():
    global _NC_CACHE
    if _NC_CACHE is None:
        _NC_CACHE = _build()
    return _NC_CACHE


def _pack_inputs(a1, v1, w_gate, b_gate, g_gate, be_gate,
                 w_val, b_val, g_val, be_val,
                 w_attn, b_attn, g_attn, be_attn,
                 w_key, b_key, g_key, be_key):
    f32 = np.float32
    a1 = np.asarray(a1, f32)
    v1 = np.asarray(v1, f32)
    # full-channel tensors (replicated)
    v1f = np.ascontiguousarray(v1.transpose(1, 0, 2).reshape(CA, N * TV))
    wa2 = np.asarray(w_attn, f32).reshape(CA, H)
    ba2 = np.asarray(b_attn, f32).reshape(CA, H)
    ga2 = np.asarray(g_attn, f32).reshape(CA, H)
    bea2 = np.asarray(be_attn, f32).reshape(CA, H)
    pcol = np.concatenate(
        [np.asarray(w_key, f32)[:, None], np.asarray(b_key, f32)[:, None],
         wa2, ba2], axis=1)
    cb1 = np.concatenate([v1f, pcol, np.ones((CA, 1), f32)], axis=1)
    cb1 = np.ascontiguousarray(cb1)
    id16 = np.eye(CL, dtype=f32)

    in_maps = []
    for i in range(NCORE):
        sl = slice(i * CL, (i + 1) * CL)
        x = a1[:, sl].reshape(B, CL, TV, RP, FQ)
        x = np.ascontiguousarray(x.transpose(0, 2, 1, 3, 4))
        a1s = x.reshape(128, AFREE)
        v1l = np.ascontiguousarray(
            v1[:, sl].transpose(1, 0, 2).reshape(CL, N * TV))
        ga4 = ga2[sl] * 0.25
        ploc = np.concatenate(
            [np.asarray(w_key, f32)[sl, None], np.asarray(b_key, f32)[sl, None],
             np.asarray(g_key, f32)[sl, None], np.asarray(be_key, f32)[sl, None],
             wa2[sl], ba2[sl], ga4,
             ga4.sum(1, keepdims=True),
             (bea2[sl] * 0.25).sum(1, keepdims=True)], axis=1)
        cb2 = np.ascontiguousarray(
            np.concatenate([v1l, ploc, id16], axis=1))
        prow = np.concatenate(
            [np.asarray(w_val, f32)[sl], np.asarray(g_val, f32)[sl],
             np.asarray(be_val, f32)[sl], np.asarray(w_gate, f32)[sl],
             np.asarray(g_gate, f32)[sl],
             np.asarray(be_gate, f32)[sl]])[None, :]
        cb3 = np.ascontiguousarray(
            np.concatenate([np.ones((1, 128), f32), prow], axis=1))
        in_maps.append({"a1s": a1s, "cb1": cb1, "cb2": cb2, "cb3": cb3})
    return in_maps


def _unpack_output(results):
    out = np.empty((N, CA, T, FQ), np.float32)
    for i in range(NCORE):
        r = np.asarray(results[i]["out"]).reshape(B, TV, CL, NS, RP, FQ)
        r = r.transpose(0, 3, 2, 1, 4, 5).reshape(N, CL, T, FQ)
        out[:, i * CL:(i + 1) * CL] = r
    return out


def _install_ntff_shim():
    """The agent image's ``antenv`` lacks ``axon_hooks``; recreate it and
    register the ctypes NTFF hook against /opt/axon/libaxon_pjrt.so (the
    same mechanism trn_boot uses when the module exists)."""
    import sys
    import types
    import ctypes
    import contextlib

    if "antenv.axon_hooks" in sys.modules:
        return True
    so_path = "/opt/axon/libaxon_pjrt.so"
    try:
        lib = ctypes.CDLL(so_path)
    except OSError:
        return False
    if not hasattr(lib, "axon_start_nrt_profile"):
        return False
    lib.axon_start_nrt_profile.argtypes = [ctypes.POINTER(ctypes.c_int64),
                                           ctypes.c_size_t]
    lib.axon_start_nrt_profile.restype = ctypes.c_int64
    lib.axon_stop_nrt_profile.argtypes = [ctypes.c_char_p]
    lib.axon_stop_nrt_profile.restype = ctypes.c_int64

    @contextlib.contextmanager
    def _hook(output_dir, device_ids):
        import jax
        jax.devices()
        if device_ids:
            ids = (ctypes.c_int64 * len(device_ids))(*device_ids)
            rc = lib.axon_start_nrt_profile(ids, len(device_ids))
        else:
            rc = lib.axon_start_nrt_profile(None, 0)
        if rc != 0:
            raise RuntimeError(f"axon_start_nrt_profile rc={rc}")
        try:
            yield
        finally:
            n = lib.axon_stop_nrt_profile(str(output_dir).encode())
            print(f"profile: {n} file(s) written to {output_dir}",
                  file=sys.stderr)

    mod = types.ModuleType("antenv.axon_hooks")
    _state = {"hook": _hook}
    mod.get_axon_ntff_profile_hook = lambda: _state["hook"]

    def set_axon_ntff_profile_hook(h):
        _state["hook"] = h

    mod.set_axon_ntff_profile_hook = set_axon_ntff_profile_hook
    import antenv
    antenv.axon_hooks = mod
    sys.modules["antenv.axon_hooks"] = mod
    return True


def run(inputs, trace=False, **trace_kwargs):
    """Returns (output, BassKernelResults)."""
    nc = _get_nc()
    in_maps = _pack_inputs(**inputs)
    if trace and not _install_ntff_shim():
        trace = False
    br = run_bass_kernel_spmd(nc, in_maps, core_ids=list(range(NCORE)),
                              trace=trace, **trace_kwargs)
    return _unpack_output(br.results), br


def kernel(**inputs):
    out, _ = run(inputs)
    return out
